# revision 37
# baseline (speedup 1.0000x reference)
"""Contrastive loss (CPC-style) kernel for 8x Trainium2 NeuronCores.

Math: reference computes, for each (step i, time t, sample s), logits over
C=33 targets (1 positive + 32 sampled negatives), then sums
-log_softmax(logits)[0] over all rows.

Reformulation (same as the verified baseline):
  With u = t + i, the 33 gather indices for a row depend only on (s, u),
  not on i.  Encode them as a multiplicity mask M[s, u, v] (counts, incl.
  the positive at v == u).  Then for each (s, u, i):
    logsumexp_row = log( sum_v M[s,u,v] * exp(H[s,u,i,v] - 80) ) + 80
  where H[s,u,i,v] = <pred[s, u-i, :, i], TL[s, v, :]> is a shifted Gram
  matrix computed densely on the tensor engine.  The positive-logit sum:
    sum_rows H[u,i,u] = sum_{u,f} (sum_i pred[s,u-i,f,i]) * TL[s,u,f]
  Rows with u < i have pred slice zero-padded -> contribute exactly
  log(33) each; corrected analytically on the host.

Performance design (201.8us baseline sim -> 110.4us; HW rel err 2.1e-4):
  * Gram matmuls in fp8e4 DoubleRow mode: 2 calls per 128x512 H tile with
    k=256 each instead of 4 bf16 calls.  DoubleRow requires the kc-pair
    stride in the weights AP to be 16-byte aligned (hence ROWB=528).
    Offline numpy check of the full fp8 pipeline: rel err 7e-4 vs the
    2e-2 gate; measured on HW: 5.4e-5.
  * all inputs pre-transposed AND pre-cast on host (fp8/bf16), shipped
    with partition-contiguous >=2KB DMA lines through HWDGE (SP queue),
    critical-prefix first; no cast-on-DMA through Pool SWDGE, no on-chip
    pad memsets (pads baked into the DRAM layout).
  * exp batched: one ACT instruction per 4 PSUM banks [128, 4*512] f32,
    double-buffered (2 x 4 banks = all 8), amortizing ACT overhead.
    ACT (~91us busy) is the irreducible exp cost of the dense approach.
  * mask-multiply + row-sum (Pool on TRN2 cannot run TensorScalarPtr or
    free-axis reduces; DVE TensorReduce/stt get no fast mode, but
    TensorTensor gets 2x and TensorScalar-with-accum gets the 4x path):
      - mask-mult: one broadcast tensor_tensor per batch, mostly on DVE
        (1127ns/batch, 2x mode), every third batch on Pool (4063ns) to
        keep DVE under the ACT pace
      - row-sum: per-step tensor_scalar (mult 1.0, add 0.0) with
        accum_out, 194ns/tile on the DVE 4x fast path -- this is what
        freed ~26us of DVE vs fused scalar_tensor_tensor
  * pipeline shaping: the first uc splits its first batch into 2-step
    mini-batches (first exp ~3us earlier during DMA fill); SE columns
    DMA out per-sample so only the last chunk rides the drain
  * no on-device Ln at all: raw SE sums (f32) are DMA'd out and the host
    takes log in float64 (exact, and drops the 3-zone Ln-blend).
"""

import numpy as np
import ml_dtypes

S, T, F = 32, 512, 512
NEG, STEPS = 32, 12
C = NEG + 1
NCORES = 8
SLOC = S // NCORES          # samples per core
KC = F // 128               # contraction chunks
UC = T // 128               # output-row chunks
PAD = STEPS                 # zero columns in front of each kc row block
ROWB = T + 16               # padded row block length (528): front PAD=12 plus
                            # 4 tail zeros -- DoubleRow Ldweights requires the
                            # kc-pair stride to be 16-byte aligned
SHIFT = 80.0
NBATCH = 4                  # i-tiles per PSUM batch (4 banks)

_CACHE: dict = {}


def _split_multi_waits(nc, max_waits: int = 1):
    """This container's walrus accepts at most one sync-wait command per
    instruction; Tile emits more.  Split extras into single-wait NoOps
    preceding the instruction on the same (serial) engine."""
    import concourse.mybir as mybir

    n_split = 0
    for fn in nc.m.functions:
        for bb in fn.blocks:
            new_insts = []
            for inst in bb.instructions:
                si = inst.sync_info
                waits = list(si.on_wait) if si is not None and si.on_wait else []
                if len(waits) > max_waits:
                    head, keep = waits[:-max_waits], waits[-max_waits:]
                    for k, w in enumerate(head):
                        nop = mybir.InstNoOp(
                            name=f"{inst.name}-w{k}",
                            engine=inst.engine,
                            ins=[],
                            outs=[],
                            sync_info=mybir.SyncInfo(on_wait=[w], on_update=[]),
                        )
                        new_insts.append(nop)
                        n_split += 1
                    inst.sync_info = mybir.SyncInfo(
                        on_wait=keep, on_update=list(si.on_update or [])
                    )
                new_insts.append(inst)
            bb.instructions = new_insts
    return n_split


def _build_bass():
    import concourse.bass as bass
    import concourse.mybir as mybir
    from concourse.tile import TileContext

    f32 = mybir.dt.float32
    bf16 = mybir.dt.bfloat16
    f8 = mybir.dt.float8e4
    ALU = mybir.AluOpType
    AF = mybir.ActivationFunctionType
    DR = mybir.MatmulPerfMode.DoubleRow  # noqa: F841
    DRSW = mybir.MatmulPerfMode.DoubleRowSwInterleave

    nc = bass.Bass()
    # host-prepped layouts (partition dim second-from... = index 1 already 128):
    #   predt8[sl, i, p, kc, x] fp8, x in [0,ROWB): x<PAD zeros, else
    #       pred[s, x-PAD, kc*128+p, i]
    predt8 = nc.dram_tensor("predt8", [SLOC, STEPS, 128, 2, 2 * ROWB], f8,
                            kind="ExternalInput")
    tlt8 = nc.dram_tensor("tlt8", [SLOC, 128, KC, T], f8, kind="ExternalInput")
    tlt16 = nc.dram_tensor("tlt16", [SLOC, 128, KC, T], bf16, kind="ExternalInput")
    msk16 = nc.dram_tensor("msk16", [SLOC, 128, UC, T], bf16, kind="ExternalInput")
    ps16 = nc.dram_tensor("ps16", [SLOC, 128, KC, T], bf16, kind="ExternalInput")
    out_se = nc.dram_tensor("out_se", [128, SLOC * UC * STEPS], f32,
                            kind="ExternalOutput")
    out_pos = nc.dram_tensor("out_pos", [128, SLOC], f32, kind="ExternalOutput")

    NB = STEPS // NBATCH  # 3 PSUM batches per (sl, uc)

    with TileContext(nc) as tc:
        with (
            tc.tile_pool(name="pt_pool", bufs=2) as pt_pool,
            tc.tile_pool(name="in_pool", bufs=2) as in_pool,
            tc.tile_pool(name="eh_pool", bufs=10) as eh_pool,
            tc.tile_pool(name="pr_pool", bufs=6) as pr_pool,
            tc.tile_pool(name="lg_pool", bufs=2) as lg_pool,
            tc.tile_pool(name="acc_pool", bufs=1) as acc_pool,
            tc.tile_pool(name="psum", bufs=2, space="PSUM") as psum_pool,
        ):
            se_all = acc_pool.tile([128, SLOC * UC * STEPS], f32)
            pos_parts = acc_pool.tile([128, SLOC], f32)
            bias_t = acc_pool.tile([128, 1], f32)
            nc.vector.memset(bias_t[:], -SHIFT)

            for sl in range(SLOC):
                # order: operands needed first (tlt8/msk), then predictions in
                # 3 chunks so the first uc/batch can start early, then the
                # positive-path operands only needed at end of the sl
                # DMA transfers share one device; order them so the minimal
                # prefix for the first batch (tlt8, msk, pt chunk 0) lands
                # first, and the positive-path operands (only needed at the
                # end of the sl) come last.
                t8 = in_pool.tile([128, KC * T], f8, tag="t8")
                t8v = t8.rearrange("p (kc v) -> p kc v", v=T)
                nc.sync.dma_start(out=t8v[:, 0:2], in_=tlt8[sl, :, 0:2])
                nc.sync.dma_start(out=t8v[:, 2:4], in_=tlt8[sl, :, 2:4])
                pt = pt_pool.tile([128, STEPS * KC * ROWB], f8, tag="pt")
                pt4 = pt.rearrange("p (i cp y) -> p i cp y", cp=2, y=2 * ROWB)
                if sl == 0:
                    nc.sync.dma_start(out=pt4[:, 0:2], in_=predt8[sl, 0:2])
                    nc.sync.dma_start(out=pt4[:, 2:4], in_=predt8[sl, 2:4])
                else:
                    nc.sync.dma_start(out=pt4[:, 0:4], in_=predt8[sl, 0:4])
                msk_t = in_pool.tile([128, UC * T], bf16, tag="msk")
                nc.sync.dma_start(
                    out=msk_t.rearrange("p (uc v) -> p uc v", v=T), in_=msk16[sl])
                for ic in range(1, 3):
                    nc.sync.dma_start(
                        out=pt4[:, 4 * ic : 4 * ic + 4],
                        in_=predt8[sl, 4 * ic : 4 * ic + 4],
                    )
                t16 = in_pool.tile([128, KC * T], bf16, tag="t16")
                nc.sync.dma_start(
                    out=t16.rearrange("p (kc v) -> p kc v", v=T), in_=tlt16[sl])
                ps_t = in_pool.tile([128, KC * T], bf16, tag="ps")
                nc.sync.dma_start(
                    out=ps_t.rearrange("p (kc u) -> p kc u", u=T), in_=ps16[sl])

                t8_3 = t8.rearrange("p (kc v) -> p kc v", v=T)

                for uc in range(UC):
                    # the very first (sl0, uc0) tile splits its first batch in
                    # two 2-step mini-batches so the first exp fires ~3us
                    # earlier while DMA still streams
                    if sl == 0 and uc == 0:
                        specs = [(0, 2), (2, 2), (4, 4), (8, 4)]
                    elif sl == SLOC - 1 and uc == UC - 1:
                        # fine-grained drain: small batches keep DVE close on
                        # ACT's heels; two mults go to the (idle) Pool
                        specs = [(0, 2), (2, 2), (4, 2), (6, 2), (8, 2), (10, 2)]
                    else:
                        specs = [(0, 4), (4, 4), (8, 4)]
                    for i0, nb in specs:
                        # mask-mult engine: mostly DVE (2x-mode broadcast
                        # tensor_tensor); one batch in three goes to Pool to
                        # keep DVE under the ACT exp pace.  Reduce: per-i
                        # tensor_scalar+accum on DVE (4x fast path, 194ns).
                        if sl == SLOC - 1 and uc == UC - 1:
                            use_pool = i0 in (2, 6)
                        else:
                            use_pool = i0 == 4 and not (sl == 0 and uc == 0)
                        ps_big = psum_pool.tile([128, nb * T], f32, tag="mm")
                        for k in range(nb):
                            i = i0 + k
                            off = PAD + uc * 128 - i
                            base = 2 * (ROWB - 128 - off)
                            for cp in range(2):
                                w = pt4[:, i, cp, base : base + 256].rearrange(
                                    "p (r m) -> p r m", m=128)
                                nc.tensor.matmul(
                                    ps_big[:, k * T : (k + 1) * T],
                                    w,
                                    t8_3[:, 2 * cp : 2 * cp + 2, :],
                                    start=(cp == 0),
                                    stop=(cp == 1),
                                    perf_mode=DRSW,
                                )
                        eh = eh_pool.tile([128, nb * T], bf16, tag="eh")
                        nc.scalar.activation(eh[:], ps_big[:], AF.Exp, bias=bias_t[:])
                        c0 = (sl * UC + uc) * STEPS + i0
                        mcol3 = msk_t.rearrange(
                            "p (uc v) -> p uc v", v=T)[:, uc : uc + 1, :]
                        prb = pr_pool.tile([128, nb * T], bf16, tag="prb")
                        pr3 = prb.rearrange("p (k v) -> p k v", v=T)
                        eh3 = eh.rearrange("p (k v) -> p k v", v=T)
                        eng = nc.gpsimd if use_pool else nc.vector
                        eng.tensor_tensor(
                            pr3, eh3, mcol3.broadcast_to([128, nb, T]),
                            ALU.mult)
                        for k in range(nb):
                            scr = pr_pool.tile([128, T], bf16, tag="scr", bufs=6)
                            nc.vector.tensor_scalar(
                                scr[:], pr3[:, k, :], 1.0, 0.0,
                                op0=ALU.mult, op1=ALU.add,
                                accum_out=se_all[:, c0 + k : c0 + k + 1])

                    if uc == 2:
                        # positive-logit partial: sum_f tlt * ps per partition
                        # row (multiply on Pool, single 4x tensor_scalar
                        # reduce on DVE).  Emitted mid-sample so Pool's
                        # in-order queue runs it before the drain tail.
                        posprod = pr_pool.tile(
                            [128, KC * T], bf16, tag="posprod", bufs=2)
                        nc.gpsimd.tensor_tensor(
                            posprod[:], t16[:], ps_t[:], ALU.mult)
                        pscr = pr_pool.tile([128, KC * T], bf16, tag="pscr", bufs=2)
                        nc.vector.tensor_scalar(
                            pscr[:], posprod[:], 1.0, 0.0,
                            op0=ALU.mult, op1=ALU.add,
                            accum_out=pos_parts[:, sl : sl + 1])

                # ship this sample's SE columns while later samples compute;
                # only the last 48-col chunk rides the drain tail
                lo = sl * UC * STEPS
                hi = (sl + 1) * UC * STEPS
                nc.sync.dma_start(
                    out=out_se[:, lo:hi], in_=se_all[:, lo:hi])

            nc.sync.dma_start(out=out_pos[:, :], in_=pos_parts[:])

    _split_multi_waits(nc)
    return nc


def _get_nc():
    if "nc" not in _CACHE:
        _CACHE["nc"] = _build_bass()
    return _CACHE["nc"]


def _prepare_inputs(true_latent, predictions, neg_indices):
    bf = ml_dtypes.bfloat16
    f8 = ml_dtypes.float8_e4m3
    tl = np.asarray(true_latent, np.float32)
    pred = np.asarray(predictions, np.float32)
    ni = np.asarray(neg_indices)

    # predt8[s, i, p, kc, PAD + t] = pred[s, t, kc*128 + p, i]
    tmp = pred.transpose(0, 3, 2, 1)                    # (S, STEPS, F, T)
    tmp = tmp.reshape(S, STEPS, KC, 128, T).transpose(0, 1, 3, 2, 4)
    ptpad = np.zeros((S, STEPS, 128, KC, ROWB), np.float32)
    ptpad[..., PAD : PAD + T] = tmp
    predt8 = np.zeros((S, STEPS, 128, 2, 2 * ROWB), f8)
    for cp in range(2):
        predt8[..., cp, 0::2] = ptpad[..., 2 * cp, ::-1].astype(f8)
        predt8[..., cp, 1::2] = ptpad[..., 2 * cp + 1, ::-1].astype(f8)

    # tlt[s, p, kc, v] = tl[s, v, kc*128 + p]
    tltm = tl.transpose(0, 2, 1).reshape(S, KC, 128, T).transpose(0, 2, 1, 3)
    tlt8 = np.ascontiguousarray(tltm.astype(f8))
    tlt16 = np.ascontiguousarray(tltm.astype(bf))

    # multiplicity mask M[s, u, v] (positive at v==u plus 32 negatives)
    j = np.arange(NEG * T)
    idx2 = ni + (ni >= (j // NEG)[None, :])
    msk = np.zeros((S, T, T), np.float32)
    rows = np.tile(np.arange(T), NEG)
    for s in range(S):
        np.add.at(msk[s], (rows, idx2[s]), 1.0)
    msk += np.eye(T, dtype=np.float32)[None]
    msk16 = np.ascontiguousarray(
        msk.reshape(S, UC, 128, T).transpose(0, 2, 1, 3).astype(bf))

    # ps[s, f, u] = sum_i pred[s, u-i, f, i]  (zero-padded shift sum)
    predt = pred.transpose(0, 3, 2, 1)                  # (S, STEPS, F, T)
    ps = np.zeros((S, F, T), np.float32)
    for i in range(STEPS):
        ps[:, :, i:] += predt[:, i, :, : T - i]
    ps16 = np.ascontiguousarray(
        ps.reshape(S, KC, 128, T).transpose(0, 2, 1, 3).astype(bf))

    in_maps = []
    for c in range(NCORES):
        lo, hi = c * SLOC, (c + 1) * SLOC
        in_maps.append(
            {
                "predt8": predt8[lo:hi],
                "tlt8": tlt8[lo:hi],
                "tlt16": tlt16[lo:hi],
                "msk16": msk16[lo:hi],
                "ps16": ps16[lo:hi],
            }
        )
    return in_maps


def _combine(results):
    lse = 0.0
    pos = 0.0
    for r in results:
        se = np.asarray(r["out_se"], np.float64)
        lse += np.log(se).sum()
        pos += np.asarray(r["out_pos"], np.float64).sum()
    n_rows = S * T * STEPS
    n_invalid = S * (STEPS * (STEPS - 1) // 2)
    loss = (lse + SHIFT * n_rows) - pos - n_invalid * np.log(C)
    return np.array([loss], np.float32)


def kernel(true_latent, predictions, neg_indices, **run_kwargs):
    from concourse.bass_utils import run_bass_kernel_spmd

    nc = _get_nc()
    in_maps = _prepare_inputs(true_latent, predictions, neg_indices)
    res = run_bass_kernel_spmd(nc, in_maps, core_ids=list(range(NCORES)), **run_kwargs)
    out = _combine(res.results)
    if run_kwargs:
        _CACHE["last_result"] = res
    return out
